# revision 1
# baseline (speedup 1.0000x reference)
"""Distributed 2-layer GCN + mean-pool + linear classifier on 8 TRN2 NeuronCores.

Strategy (sharding hint: partition nodes + incident edges across cores):
  - Nodes are range-partitioned across the 8 cores (12500 real, padded 12544).
  - Each core owns the edges whose *destination* lies in its node range, so
    the scatter side of message passing is core-local.
  - Per GCN layer, each core computes g = D^-1/2 (h @ W) for its nodes on the
    TensorEngine, the g shards are AllGathered (split into 4 sub-collectives
    so gather tables stay int16-indexable), and each core pulls the source
    rows of its edges with dma_gather (one 512B descriptor per edge).
  - The per-destination reduction runs on the TensorEngine: tokens are
    pre-sorted by destination tile, a one-hot indicator is built on the
    VectorEngine (iota + is_equal against the per-token dst id) and
    matmul-accumulated into PSUM, one accumulation group per (quarter,
    dst-tile) run. Self-loop terms are folded in as the accumulator init.
  - deg^-1/2 scaling both pre-gather (src side) and post-aggregation (dst
    side) makes the edge weight dinv[s]*dinv[d] without any per-edge floats.
  - Mean-pool runs as one more indicator matmul into a [feat, graph] PSUM
    tile plus a count matmul, a [128,129] AllReduce, and the final linear
    is computed redundantly on every core.

All heavy traffic (feature gathers) is HBM-bandwidth-bound; host-side work is
restricted to integer index bucketing/sorting and degree/count histograms.
"""
import numpy as np

from concourse import bacc, bass, mybir, tile
from concourse.bass_utils import run_bass_kernel_spmd
from concourse.masks import make_identity

F32 = mybir.dt.float32
BF16 = mybir.dt.bfloat16
I16 = mybir.dt.int16

N_NODES = 100000
N_EDGES = 1600000
N_CORES = 8
F = 128
G = 128
OD = 16


def default_cfg():
    return dict(
        n=N_NODES, cores=N_CORES,
        nsh=N_NODES // N_CORES,      # real nodes per core
        nt=98,                       # node tiles per core (padded)
        q=4,                         # gather-table quarters
        ch_tiles=4,                  # token-tiles per gather chunk (512 tokens =
                                     # 33 descs/engine: ~4 instructions resident
                                     # in the SWDGE ring keeps DGE streaming)
        bf16=0,                      # bf16 gather tables / messages / agg matmul
        queues=1,                    # SWDGE queues to spread gathers over
        seg16=1,                     # 16-token run padding + segment indicators
    )


def _segments(runs):
    """Split (D, ntokens) runs into per-128-token-tile segments.

    Returns (segs, by_tile): segs[i] = (tile, D, first, last); by_tile maps
    tile -> list of (col, D, first, last) with col = index into segs (the
    relseg column). Shared by host_prep and build_program so the relseg
    stream layout and the emitted matmul schedule always agree.
    """
    segs = []
    pos = 0
    for (D, ntok_r) in runs:
        a, b = pos, pos + ntok_r
        for k in range(a // 128, (b + 127) // 128):
            s0, s1 = max(a, k * 128), min(b, (k + 1) * 128)
            segs.append((k, D, s0 == a, s1 == b, s0, s1))
        pos = b
    by_tile = {}
    for col, (k, D, fi, la, _s0, _s1) in enumerate(segs):
        by_tile.setdefault(k, []).append((col, D, fi, la))
    return segs, by_tile


def _derived(cfg):
    npad = cfg["nt"] * 128
    qsh = npad // cfg["q"]
    tbl = cfg["cores"] * qsh
    assert npad % cfg["q"] == 0 and tbl <= 32768
    return npad, qsh, tbl


# ----------------------------------------------------------------- host prep

def host_prep(x, edge_index, batch, cfg):
    n, C, NSH, NT, Q = cfg["n"], cfg["cores"], cfg["nsh"], cfg["nt"], cfg["q"]
    NPAD, QSH, TBL = _derived(cfg)
    src = np.asarray(edge_index[0], dtype=np.int64)
    dst = np.asarray(edge_index[1], dtype=np.int64)
    batch = np.asarray(batch, dtype=np.int64)
    x = np.asarray(x, dtype=np.float32)

    deg = np.bincount(dst, minlength=n).astype(np.float32) + 1.0  # + self loop

    so = src // NSH
    r = src - so * NSH
    sq = r // QSH
    trow = so * QSH + (r - sq * QSH)

    down = dst // NSH
    dslot = dst - down * NSH
    dtile = dslot // 128
    drel = dslot - dtile * 128

    NQD = Q * NT
    seg_all = sq * NT + dtile
    counts = np.zeros((C, NQD), dtype=np.int64)
    for c in range(C):
        counts[c] = np.bincount(seg_all[down == c], minlength=NQD)

    grain = 16 if cfg.get("seg16") else 128
    caps = (counts.max(axis=0) + grain - 1) // grain * grain  # tokens per (q,D)
    for q in range(Q):
        if caps[q * NT:(q + 1) * NT].sum() == 0:
            caps[q * NT] = grain  # keep every per-quarter stream non-empty
    base = np.zeros(NQD, dtype=np.int64)
    np.cumsum(caps[:-1], out=base[1:])
    Ltot = int(caps.sum())
    q_start = np.zeros(Q + 1, dtype=np.int64)
    for q in range(Q):
        q_start[q + 1] = q_start[q] + int(caps[q * NT:(q + 1) * NT].sum())

    sched = []
    for q in range(Q):
        sched.append([(D, int(caps[q * NT + D])) for D in range(NT)
                      if caps[q * NT + D] > 0])
    # padded per-quarter stream lengths (tail-padded to full token tiles)
    Lqs = [int(-(-(q_start[q + 1] - q_start[q]) // 128) * 128) for q in range(Q)]
    seg_cols = [_segments(sched[q])[0] for q in range(Q)]

    per_core = []
    for c in range(C):
        m = down == c
        tr_c, seg_c, rel_c = trow[m], seg_all[m], drel[m]
        order = np.lexsort((tr_c, seg_c))
        tr_c, seg_c, rel_c = tr_c[order], seg_c[order], rel_c[order]
        starts = np.zeros(NQD, dtype=np.int64)
        np.cumsum(counts[c][:-1], out=starts[1:])
        pos = base[seg_c] + (np.arange(len(seg_c)) - starts[seg_c])
        tr_pad = np.zeros(Ltot, dtype=np.int16)
        rel_pad = np.full(Ltot, -1.0, dtype=np.float32)
        tr_pad[pos] = tr_c.astype(np.int16)
        rel_pad[pos] = rel_c.astype(np.float32)

        idx_arrs, rel_arrs = [], []
        for q in range(Q):
            sl = slice(q_start[q], q_start[q + 1])
            Lq = int(q_start[q + 1] - q_start[q])
            tr_q = np.zeros(Lqs[q], dtype=np.int16)
            tr_q[:Lq] = tr_pad[sl]
            iw = tr_q.reshape(Lqs[q] // 16, 16).T
            idx_arrs.append(np.tile(iw, (8, 1)).copy())          # [128, Lq/16] i16
            rel_q = rel_pad[sl]
            cols = np.full((128, len(seg_cols[q])), -1.0, dtype=np.float32)
            for col, (k, D, fi, la, s0, s1) in enumerate(seg_cols[q]):
                cols[s0 - 128 * k:s1 - 128 * k, col] = rel_q[s0:s1]
            rel_arrs.append(cols)                                # [128, ncols] f32

        x_c = np.zeros((NPAD, F), dtype=np.float32)
        x_c[:NSH] = x[c * NSH:(c + 1) * NSH]
        deg_c = np.ones(NPAD, dtype=np.float32)
        deg_c[:NSH] = deg[c * NSH:(c + 1) * NSH]
        bat_c = np.full(NPAD, -1.0, dtype=np.float32)
        bat_c[:NSH] = batch[c * NSH:(c + 1) * NSH]
        per_core.append(dict(
            x=x_c,
            deg=deg_c.reshape(NT, 128).T.copy(),
            bat=bat_c.reshape(NT, 128).T.copy(),
            idx=idx_arrs, rel=rel_arrs,
        ))

    return per_core, sched, Lqs


# ------------------------------------------------------------ program build

def build_program(cfg, sched, Lqs, b1, b2, bc, bench=None):
    bench = bench or {}
    n_repeats = bench.get("repeats", 1)
    no_ag = bench.get("no_ag", False)
    no_gather = bench.get("no_gather", False)
    no_ind = bench.get("no_ind", False)
    no_mm = bench.get("no_mm", False)
    C, NT, Q, CHT = cfg["cores"], cfg["nt"], cfg["q"], cfg["ch_tiles"]
    NPAD, QSH, TBL = _derived(cfg)
    CH = CHT * 128
    GDT = BF16 if cfg.get("bf16") else F32
    NQUEUES = int(cfg.get("queues", 1))
    rg = [list(range(C))]
    use_b1 = bool(np.any(b1)); use_b2 = bool(np.any(b2)); use_bc = bool(np.any(bc))

    nc = bacc.Bacc("TRN2", target_bir_lowering=False, debug=False,
                   num_devices=C, num_swdge_queues=max(1, NQUEUES))

    x_d = nc.dram_tensor("x", [NPAD, F], F32, kind="ExternalInput")
    deg_d = nc.dram_tensor("deg", [128, NT], F32, kind="ExternalInput")
    bat_d = nc.dram_tensor("bat", [128, NT], F32, kind="ExternalInput")
    w1_d = nc.dram_tensor("w1", [F, F], F32, kind="ExternalInput")
    w2_d = nc.dram_tensor("w2", [F, F], F32, kind="ExternalInput")
    wc_d = nc.dram_tensor("wc", [F, OD], F32, kind="ExternalInput")
    b1_d = nc.dram_tensor("b1b", [128, F], F32, kind="ExternalInput") if use_b1 else None
    b2_d = nc.dram_tensor("b2b", [128, F], F32, kind="ExternalInput") if use_b2 else None
    bc_d = nc.dram_tensor("bcb", [128, OD], F32, kind="ExternalInput") if use_bc else None
    idx_d = [nc.dram_tensor(f"idx{q}", [128, Lqs[q] // 16], I16,
                            kind="ExternalInput") for q in range(Q)]
    seg_info = [_segments(sched[q]) for q in range(Q)]
    rel_d = [nc.dram_tensor(f"rel{q}", [128, len(seg_info[q][0])], F32,
                            kind="ExternalInput") for q in range(Q)]
    out_d = nc.dram_tensor("out", [G, OD], F32, kind="ExternalOutput")

    # per-quarter bounce tensors: keeps the AllGather's read dependency
    # narrow so AG_q can fire as soon as its own 25 node tiles are done
    gb_d = [nc.dram_tensor(f"gbounce{q}", [QSH, F], GDT) for q in range(Q)]
    tbl_d = [nc.dram_tensor(f"tbl{q}", [TBL, F], GDT, addr_space="Shared")
             for q in range(Q)]
    ar_in_d = nc.dram_tensor("ar_in", [128, F + 1], F32)
    ar_out_d = nc.dram_tensor("ar_out", [128, F + 1], F32, addr_space="Shared")

    with tile.TileContext(nc) as tc:
        with (
            tc.tile_pool(name="stat", bufs=1) as stat,
            tc.tile_pool(name="hA", bufs=NT) as poolA,
            tc.tile_pool(name="hB", bufs=NT) as poolB,
            tc.tile_pool(name="msg", bufs=int(cfg.get("bufs", 3))) as poolM,
            tc.tile_pool(name="ind", bufs=6) as poolI,
            tc.tile_pool(name="ldx", bufs=int(cfg.get("bufs", 3))) as poolL,
            tc.tile_pool(name="pt", bufs=1, space=bass.MemorySpace.PSUM) as pp_t,
            tc.tile_pool(name="pg", bufs=1, space=bass.MemorySpace.PSUM) as pp_g,
            tc.tile_pool(name="pa", bufs=4, space=bass.MemorySpace.PSUM) as pp_a,
            tc.tile_pool(name="pp", bufs=1, space=bass.MemorySpace.PSUM) as pp_p,
        ):
            # ------- static tiles
            ident = stat.tile([128, 128], F32, name="ident", tag="ident")
            make_identity(nc, ident[:])
            iota = stat.tile([128, 128], GDT, name="iota", tag="iota")
            nc.gpsimd.iota(iota[:], pattern=[[1, 128]], base=0,
                           channel_multiplier=0,
                           allow_small_or_imprecise_dtypes=True)
            w1_s = stat.tile([F, F], F32, name="w1s", tag="w1s")
            nc.sync.dma_start(w1_s[:], w1_d[:])
            w2_s = stat.tile([F, F], F32, name="w2s", tag="w2s")
            nc.sync.dma_start(w2_s[:], w2_d[:])
            wc_s = stat.tile([F, OD], F32, name="wcs", tag="wcs")
            nc.sync.dma_start(wc_s[:], wc_d[:])
            deg_s = stat.tile([128, NT], F32, name="degs", tag="degs")
            nc.sync.dma_start(deg_s[:], deg_d[:])
            bat_s = stat.tile([128, NT], F32, name="bats", tag="bats")
            nc.sync.dma_start(bat_s[:], bat_d[:])
            sqd = stat.tile([128, NT], F32, name="sqd", tag="sqd")
            nc.scalar.sqrt(sqd[:], deg_s[:])
            dinv = stat.tile([128, NT], F32, name="dinv", tag="dinv")
            nc.vector.reciprocal(dinv[:], sqd[:])
            ones = stat.tile([128, 1], F32, name="ones", tag="ones")
            nc.vector.memset(ones[:], 1.0)
            bias_s = []
            for use, bd, shape in ((use_b1, b1_d, [128, F]),
                                   (use_b2, b2_d, [128, F]),
                                   (use_bc, bc_d, [128, OD])):
                if use:
                    t = stat.tile(shape, F32, name=f"bs{len(bias_s)}",
                                  tag=f"bs{len(bias_s)}")
                    nc.sync.dma_start(t[:], bd[:])
                    bias_s.append(t)
                else:
                    bias_s.append(None)

            hA = [poolA.tile([128, F], F32, name=f"hA{j}", tag="hA")
                  for j in range(NT)]
            hB = [poolB.tile([128, F], F32, name=f"hB{j}", tag="hB")
                  for j in range(NT)]

            def layer(li, w_s, bias_t):
                # --- g = dinv * (h @ w); h source: x from DRAM (L1) or hB (L2)
                for j in range(NT):
                    if li == 0:
                        xj = poolM.tile([128, F], F32, name=f"x{li}_{j}",
                                        tag="xin")
                        nc.sync.dma_start(xj[:], x_d[j * 128:(j + 1) * 128, :])
                    else:
                        xj = hB[j]
                    ptr = pp_t.tile([128, 128], F32, name=f"tp{li}_{j}",
                                    tag="ptr")
                    nc.tensor.transpose(ptr[:], xj[:], ident[:])
                    xT = poolM.tile([128, F], F32, name=f"xT{li}_{j}", tag="xT")
                    nc.scalar.copy(xT[:], ptr[:])
                    pg = pp_g.tile([128, F], F32, name=f"pg{li}_{j}", tag="pg")
                    nc.tensor.matmul(pg[:], xT[:], w_s[:])
                    nc.scalar.activation(hA[j][:], pg[:],
                                         mybir.ActivationFunctionType.Copy,
                                         scale=dinv[:, j:j + 1])
                    if GDT is BF16:
                        src = poolM.tile([128, F], BF16, name=f"gbf{li}_{j}",
                                         tag="gbf")
                        nc.scalar.activation(src[:], pg[:],
                                             mybir.ActivationFunctionType.Copy,
                                             scale=dinv[:, j:j + 1])
                    else:
                        src = hA[j]
                    r_lo, r_hi = j * 128, j * 128 + 128
                    for q in range(r_lo // QSH, (r_hi - 1) // QSH + 1):
                        r0, r1 = max(r_lo, q * QSH), min(r_hi, (q + 1) * QSH)
                        nc.sync.dma_start(
                            gb_d[q][r0 - q * QSH:r1 - q * QSH, :],
                            src[r0 - r_lo:r1 - r_lo, :])
                # --- allgather the scaled shard into the 4 tables
                if not no_ag:
                    for q in range(Q):
                        nc.gpsimd.collective_compute(
                            "AllGather", mybir.AluOpType.bypass,
                            replica_groups=rg,
                            ins=[gb_d[q][:]],
                            outs=[tbl_d[q][:]],
                        )
                # --- gather + indicator-matmul aggregation, acc init = hA (self loop)
                for q in range(Q):
                    segs, by_tile = seg_info[q]
                    Lq = Lqs[q]
                    ntiles_q = Lq // 128
                    nchunks = (ntiles_q + CHT - 1) // CHT
                    chunk_cols = []
                    for ci in range(nchunks):
                        t0 = ci * CHT
                        nt_c = min(CHT, ntiles_q - t0)
                        cc = [c for k in range(t0, t0 + nt_c)
                              for (c, _D, _f, _l) in by_tile.get(k, [])]
                        chunk_cols.append((cc[0], cc[-1] + 1) if cc else (0, 0))
                    maxc = max(max((b - a for a, b in chunk_cols), default=1), 1)
                    cur_psum = {}
                    for ci in range(nchunks):
                        t0 = ci * CHT
                        ntile = min(CHT, ntiles_q - t0)
                        ntok = ntile * 128
                        idxt = poolL.tile([128, CH // 16], I16,
                                          name=f"ix{li}_{q}_{ci}", tag="idxt")
                        nc.sync.dma_start(
                            idxt[:, :ntok // 16],
                            idx_d[q][:, t0 * 8:t0 * 8 + ntok // 16])
                        c0, c1 = chunk_cols[ci]
                        relt = poolL.tile([128, maxc], F32,
                                          name=f"rl{li}_{q}_{ci}", tag="relt")
                        if c1 > c0:
                            nc.sync.dma_start(relt[:, :c1 - c0],
                                              rel_d[q][:, c0:c1])
                        msg = poolM.tile([128, CHT, F], GDT,
                                         name=f"mg{li}_{q}_{ci}", tag="msg")
                        if not no_gather:
                            nc.gpsimd.dma_gather(
                                msg[:, :ntile, :], tbl_d[q][:],
                                idxt[:, :ntok // 16], ntok, ntok, F,
                                queue_num=(ci % NQUEUES))
                        else:
                            # same-volume sequential DMA: ablates only the
                            # random-access/descriptor cost of the gather
                            nc.sync.dma_start(
                                msg[:, :ntile, :],
                                tbl_d[q][0:ntok, :].rearrange(
                                    "(c p) f -> p c f", p=128))
                        for k in range(t0, t0 + ntile):
                            for (col, D, first, last) in by_tile.get(k, []):
                                if no_ind and no_mm:
                                    continue
                                ind = poolI.tile([128, 128], GDT,
                                                 name=f"in{li}_{q}_{ci}_{col}",
                                                 tag="ind")
                                if not no_ind:
                                    nc.vector.tensor_scalar(
                                        ind[:], iota[:],
                                        relt[:, col - c0:col - c0 + 1],
                                        None, mybir.AluOpType.is_equal)
                                if no_mm:
                                    continue
                                if first:
                                    pa = pp_a.tile([128, F], F32,
                                                   name=f"pa{li}_{q}_{D}",
                                                   tag="pa")
                                    cur_psum[D] = pa
                                pa = cur_psum[D]
                                nc.tensor.matmul(pa[:], ind[:],
                                                 msg[:, k - t0, :],
                                                 start=first, stop=last)
                                if last:
                                    nc.vector.tensor_tensor(
                                        out=hA[D][:], in0=hA[D][:], in1=pa[:],
                                        op=mybir.AluOpType.add)
                                    del cur_psum[D]
                # --- finalize h = relu(dinv * acc (+ b))
                for j in range(NT):
                    if bias_t is None:
                        nc.scalar.activation(hB[j][:], hA[j][:],
                                             mybir.ActivationFunctionType.Relu,
                                             scale=dinv[:, j:j + 1])
                    else:
                        tmp = poolI.tile([128, F], F32, name=f"bt{li}_{j}",
                                         tag="ind")
                        nc.vector.tensor_scalar(tmp[:], hA[j][:],
                                                dinv[:, j:j + 1], None,
                                                mybir.AluOpType.mult)
                        nc.vector.tensor_tensor(out=tmp[:], in0=tmp[:],
                                                in1=bias_t[:],
                                                op=mybir.AluOpType.add)
                        nc.scalar.activation(hB[j][:], tmp[:],
                                             mybir.ActivationFunctionType.Relu)

            for _r in range(n_repeats):
                layer(0, w1_s, bias_s[0])
                layer(1, w2_s, bias_s[1])

            # ------- pooling: sums^T [feat, graph] and counts [graph, 1]
            ps = pp_p.tile([128, G], F32, name="psums", tag="psums")
            for j in range(NT):
                indg = poolI.tile([128, G], F32, name=f"ig{j}", tag="ind")
                nc.vector.tensor_scalar(indg[:], iota[:], bat_s[:, j:j + 1],
                                        None, mybir.AluOpType.is_equal)
                nc.tensor.matmul(ps[:], hB[j][:], indg[:],
                                 start=(j == 0), stop=(j == NT - 1))
            pn = pp_p.tile([128, 1], F32, name="pcnt", tag="pcnt")
            for j in range(NT):
                indg = poolI.tile([128, G], F32, name=f"ic{j}", tag="ind")
                nc.vector.tensor_scalar(indg[:], iota[:], bat_s[:, j:j + 1],
                                        None, mybir.AluOpType.is_equal)
                nc.tensor.matmul(pn[:], indg[:], ones[:],
                                 start=(j == 0), stop=(j == NT - 1))
            pack = stat.tile([128, F + 1], F32, name="pack", tag="pack")
            nc.scalar.copy(pack[:, 0:F], ps[:])
            nc.scalar.copy(pack[:, F:F + 1], pn[:])
            nc.sync.dma_start(ar_in_d[:], pack[:])
            nc.gpsimd.collective_compute(
                "AllReduce", mybir.AluOpType.add, replica_groups=rg,
                ins=[ar_in_d[:]], outs=[ar_out_d[:]])
            sums = stat.tile([128, F + 1], F32, name="sums", tag="sums")
            nc.sync.dma_start(sums[:], ar_out_d[:])
            cnt1 = stat.tile([128, 1], F32, name="cnt1", tag="cnt1")
            nc.vector.tensor_scalar_max(cnt1[:], sums[:, F:F + 1], 1.0)
            rcp = stat.tile([128, 1], F32, name="rcp", tag="rcp")
            nc.vector.reciprocal(rcp[:], cnt1[:])
            po = pp_g.tile([128, OD], F32, name="po", tag="pg")
            nc.tensor.matmul(po[:], sums[:, 0:F], wc_s[:])
            osb = stat.tile([128, OD], F32, name="osb", tag="osb")
            nc.scalar.activation(osb[:], po[:],
                                 mybir.ActivationFunctionType.Copy,
                                 scale=rcp[:])
            if bias_s[2] is not None:
                nc.vector.tensor_tensor(out=osb[:], in0=osb[:],
                                        in1=bias_s[2][:],
                                        op=mybir.AluOpType.add)
            nc.sync.dma_start(out_d[:], osb[:])

    nc.compile()
    return nc


# ------------------------------------------------------------------- driver

def run(inputs, cfg, trace=False):
    x = np.asarray(inputs["x"], dtype=np.float32)
    edge_index = np.asarray(inputs["edge_index"])
    batch = np.asarray(inputs["batch"])
    w1 = np.asarray(inputs["w1"], dtype=np.float32)
    b1 = np.asarray(inputs["b1"], dtype=np.float32)
    w2 = np.asarray(inputs["w2"], dtype=np.float32)
    b2 = np.asarray(inputs["b2"], dtype=np.float32)
    wc = np.asarray(inputs["wc"], dtype=np.float32)
    bc = np.asarray(inputs["bc"], dtype=np.float32)

    per_core, sched, Lqs = host_prep(x, edge_index, batch, cfg)
    nc = build_program(cfg, sched, Lqs, b1, b2, bc)

    in_maps = []
    for c in range(cfg["cores"]):
        pc = per_core[c]
        m = dict(x=pc["x"], deg=pc["deg"], bat=pc["bat"],
                 w1=w1, w2=w2, wc=wc)
        for q in range(cfg["q"]):
            m[f"idx{q}"] = pc["idx"][q]
            m[f"rel{q}"] = pc["rel"][q]
        if np.any(b1):
            m["b1b"] = np.tile(b1[None, :], (128, 1)).astype(np.float32)
        if np.any(b2):
            m["b2b"] = np.tile(b2[None, :], (128, 1)).astype(np.float32)
        if np.any(bc):
            m["bcb"] = np.tile(bc[None, :], (128, 1)).astype(np.float32)
        in_maps.append(m)

    res = run_bass_kernel_spmd(nc, in_maps, list(range(cfg["cores"])),
                               trace=trace)
    out = np.asarray(res.results[0]["out"], dtype=np.float32)
    return out, res


def kernel(**inputs) -> np.ndarray:
    out, _ = run(inputs, default_cfg(), trace=False)
    return out



# revision 4
# speedup vs baseline: 5.0692x; 5.0692x over previous
"""Distributed 2-layer GCN + mean-pool + linear classifier on 8 TRN2 NeuronCores.

Strategy (sharding hint: partition nodes + incident edges across cores):
  - Nodes are range-partitioned across the 8 cores (12500 real, padded 12544).
  - Each core owns the edges whose *destination* lies in its node range, so
    the scatter side of message passing is core-local.
  - Per GCN layer, each core computes g = D^-1/2 (h @ W) for its nodes on the
    TensorEngine, the g shards are AllGathered (split into 4 sub-collectives
    so gather tables stay int16-indexable), and each core pulls the source
    rows of its edges with dma_gather (one 512B descriptor per edge).
  - The per-destination reduction runs on the TensorEngine: tokens are
    pre-sorted by destination tile, a one-hot indicator is built on the
    VectorEngine (iota + is_equal against the per-token dst id) and
    matmul-accumulated into PSUM, one accumulation group per (quarter,
    dst-tile) run. Self-loop terms are folded in as the accumulator init.
  - deg^-1/2 scaling both pre-gather (src side) and post-aggregation (dst
    side) makes the edge weight dinv[s]*dinv[d] without any per-edge floats.
  - Mean-pool runs as one more indicator matmul into a [feat, graph] PSUM
    tile plus a count matmul, a [128,129] AllReduce, and the final linear
    is computed redundantly on every core.

All heavy traffic (feature gathers) is HBM-bandwidth-bound; host-side work is
restricted to integer index bucketing/sorting and degree/count histograms.
"""
import numpy as np

from concourse import bacc, bass, mybir, tile
from concourse.bass_utils import run_bass_kernel_spmd
from concourse.masks import make_identity

F32 = mybir.dt.float32
BF16 = mybir.dt.bfloat16
I16 = mybir.dt.int16

N_NODES = 100000
N_EDGES = 1600000
N_CORES = 8
F = 128
G = 128
OD = 16


def default_cfg():
    return dict(
        n=N_NODES, cores=N_CORES,
        nsh=N_NODES // N_CORES,      # real nodes per core
        nt=98,                       # node tiles per core (padded)
        q=4,                         # gather-table quarters
        ch_tiles=8,                  # token-tiles per gather chunk (1024 tokens
                                     # = one full 1024-desc SWDGE ring per
                                     # instruction; 2048 descs deadlocks ucode)
        bf16=1,                      # bf16 gather tables / messages / agg matmul
        queues=1,                    # SWDGE queues to spread gathers over
        seg16=1,                     # 16-token run padding + segment indicators
        scratch=16384,               # SWDGE ring bytes (descs = scratch/16)
    )


def _segments(runs):
    """Split (D, ntokens) runs into per-128-token-tile segments.

    Returns (segs, by_tile): segs[i] = (tile, D, first, last); by_tile maps
    tile -> list of (col, D, first, last) with col = index into segs (the
    relseg column). Shared by host_prep and build_program so the relseg
    stream layout and the emitted matmul schedule always agree.
    """
    segs = []
    pos = 0
    for (D, ntok_r) in runs:
        a, b = pos, pos + ntok_r
        for k in range(a // 128, (b + 127) // 128):
            s0, s1 = max(a, k * 128), min(b, (k + 1) * 128)
            segs.append((k, D, s0 == a, s1 == b, s0, s1))
        pos = b
    by_tile = {}
    for col, (k, D, fi, la, _s0, _s1) in enumerate(segs):
        by_tile.setdefault(k, []).append((col, D, fi, la))
    return segs, by_tile


def _derived(cfg):
    npad = cfg["nt"] * 128
    qsh = npad // cfg["q"]
    tbl = cfg["cores"] * qsh
    assert npad % cfg["q"] == 0 and tbl <= 32768
    return npad, qsh, tbl


# ----------------------------------------------------------------- host prep

def host_prep(x, edge_index, batch, cfg):
    n, C, NSH, NT, Q = cfg["n"], cfg["cores"], cfg["nsh"], cfg["nt"], cfg["q"]
    NPAD, QSH, TBL = _derived(cfg)
    src = np.asarray(edge_index[0], dtype=np.int64)
    dst = np.asarray(edge_index[1], dtype=np.int64)
    batch = np.asarray(batch, dtype=np.int64)
    x = np.asarray(x, dtype=np.float32)

    deg = np.bincount(dst, minlength=n).astype(np.float32) + 1.0  # + self loop

    so = src // NSH
    r = src - so * NSH
    sq = r // QSH
    trow = so * QSH + (r - sq * QSH)

    down = dst // NSH
    dslot = dst - down * NSH
    dtile = dslot // 128
    drel = dslot - dtile * 128

    NQD = Q * NT
    seg_all = sq * NT + dtile
    counts = np.zeros((C, NQD), dtype=np.int64)
    for c in range(C):
        counts[c] = np.bincount(seg_all[down == c], minlength=NQD)

    grain = 16 if cfg.get("seg16") else 128
    caps = (counts.max(axis=0) + grain - 1) // grain * grain  # tokens per (q,D)
    for q in range(Q):
        if caps[q * NT:(q + 1) * NT].sum() == 0:
            caps[q * NT] = grain  # keep every per-quarter stream non-empty
    base = np.zeros(NQD, dtype=np.int64)
    np.cumsum(caps[:-1], out=base[1:])
    Ltot = int(caps.sum())
    q_start = np.zeros(Q + 1, dtype=np.int64)
    for q in range(Q):
        q_start[q + 1] = q_start[q] + int(caps[q * NT:(q + 1) * NT].sum())

    sched = []
    for q in range(Q):
        sched.append([(D, int(caps[q * NT + D])) for D in range(NT)
                      if caps[q * NT + D] > 0])
    # padded per-quarter stream lengths (tail-padded to full token tiles)
    Lqs = [int(-(-(q_start[q + 1] - q_start[q]) // 128) * 128) for q in range(Q)]
    seg_cols = [_segments(sched[q])[0] for q in range(Q)]

    per_core = []
    for c in range(C):
        m = down == c
        tr_c, seg_c, rel_c = trow[m], seg_all[m], drel[m]
        order = np.lexsort((tr_c, seg_c))
        tr_c, seg_c, rel_c = tr_c[order], seg_c[order], rel_c[order]
        starts = np.zeros(NQD, dtype=np.int64)
        np.cumsum(counts[c][:-1], out=starts[1:])
        pos = base[seg_c] + (np.arange(len(seg_c)) - starts[seg_c])
        tr_pad = np.zeros(Ltot, dtype=np.int16)
        rel_pad = np.full(Ltot, -1.0, dtype=np.float32)
        tr_pad[pos] = tr_c.astype(np.int16)
        rel_pad[pos] = rel_c.astype(np.float32)

        idx_arrs, rel_arrs = [], []
        for q in range(Q):
            sl = slice(q_start[q], q_start[q + 1])
            Lq = int(q_start[q + 1] - q_start[q])
            tr_q = np.zeros(Lqs[q], dtype=np.int16)
            tr_q[:Lq] = tr_pad[sl]
            iw = tr_q.reshape(Lqs[q] // 16, 16).T
            idx_arrs.append(np.tile(iw, (8, 1)).copy())          # [128, Lq/16] i16
            rel_q = rel_pad[sl]
            cols = np.full((128, len(seg_cols[q])), -1.0, dtype=np.float32)
            for col, (k, D, fi, la, s0, s1) in enumerate(seg_cols[q]):
                cols[s0 - 128 * k:s1 - 128 * k, col] = rel_q[s0:s1]
            rel_arrs.append(cols)                                # [128, ncols] f32

        x_c = np.zeros((NPAD, F), dtype=np.float32)
        x_c[:NSH] = x[c * NSH:(c + 1) * NSH]
        deg_c = np.ones(NPAD, dtype=np.float32)
        deg_c[:NSH] = deg[c * NSH:(c + 1) * NSH]
        bat_c = np.full(NPAD, -1.0, dtype=np.float32)
        bat_c[:NSH] = batch[c * NSH:(c + 1) * NSH]
        per_core.append(dict(
            x=x_c,
            deg=deg_c.reshape(NT, 128).T.copy(),
            bat=bat_c.reshape(NT, 128).T.copy(),
            idx=idx_arrs, rel=rel_arrs,
        ))

    return per_core, sched, Lqs


# ------------------------------------------------------------ program build

def build_program(cfg, sched, Lqs, b1, b2, bc, bench=None):
    bench = bench or {}
    n_repeats = bench.get("repeats", 1)
    no_ag = bench.get("no_ag", False)
    no_gather = bench.get("no_gather", False)
    no_ind = bench.get("no_ind", False)
    no_mm = bench.get("no_mm", False)
    C, NT, Q, CHT = cfg["cores"], cfg["nt"], cfg["q"], cfg["ch_tiles"]
    NPAD, QSH, TBL = _derived(cfg)
    CH = CHT * 128
    GDT = BF16 if cfg.get("bf16") else F32
    NQUEUES = int(cfg.get("queues", 1))
    rg = [list(range(C))]
    use_b1 = bool(np.any(b1)); use_b2 = bool(np.any(b2)); use_bc = bool(np.any(bc))

    nc = bacc.Bacc("TRN2", target_bir_lowering=False, debug=False,
                   num_devices=C, num_swdge_queues=max(1, NQUEUES),
                   dynamic_dma_scratch_size=int(cfg.get("scratch", 16384)))

    x_d = nc.dram_tensor("x", [NPAD, F], F32, kind="ExternalInput")
    deg_d = nc.dram_tensor("deg", [128, NT], F32, kind="ExternalInput")
    bat_d = nc.dram_tensor("bat", [128, NT], F32, kind="ExternalInput")
    w1_d = nc.dram_tensor("w1", [F, F], F32, kind="ExternalInput")
    w2_d = nc.dram_tensor("w2", [F, F], F32, kind="ExternalInput")
    wc_d = nc.dram_tensor("wc", [F, OD], F32, kind="ExternalInput")
    b1_d = nc.dram_tensor("b1b", [128, F], F32, kind="ExternalInput") if use_b1 else None
    b2_d = nc.dram_tensor("b2b", [128, F], F32, kind="ExternalInput") if use_b2 else None
    bc_d = nc.dram_tensor("bcb", [128, OD], F32, kind="ExternalInput") if use_bc else None
    idx_d = [nc.dram_tensor(f"idx{q}", [128, Lqs[q] // 16], I16,
                            kind="ExternalInput") for q in range(Q)]
    seg_info = [_segments(sched[q]) for q in range(Q)]
    rel_d = [nc.dram_tensor(f"rel{q}", [128, len(seg_info[q][0])], F32,
                            kind="ExternalInput") for q in range(Q)]
    out_d = nc.dram_tensor("out", [G, OD], F32, kind="ExternalOutput")

    # per-quarter bounce tensors: keeps the AllGather's read dependency
    # narrow so AG_q can fire as soon as its own 25 node tiles are done
    gb_d = [nc.dram_tensor(f"gbounce{q}", [QSH, F], GDT) for q in range(Q)]
    tbl_d = [nc.dram_tensor(f"tbl{q}", [TBL, F], GDT, addr_space="Shared")
             for q in range(Q)]
    ar_in_d = nc.dram_tensor("ar_in", [128, F + 1], F32)
    ar_out_d = nc.dram_tensor("ar_out", [128, F + 1], F32, addr_space="Shared")

    with tile.TileContext(nc) as tc:
        with (
            tc.tile_pool(name="stat", bufs=1) as stat,
            tc.tile_pool(name="hA", bufs=NT) as poolA,
            tc.tile_pool(name="hB", bufs=NT) as poolB,
            tc.tile_pool(name="msg", bufs=int(cfg.get("bufs", 3))) as poolM,
            tc.tile_pool(name="ind", bufs=6) as poolI,
            tc.tile_pool(name="ldx", bufs=int(cfg.get("bufs", 3))) as poolL,
            tc.tile_pool(name="pt", bufs=1, space=bass.MemorySpace.PSUM) as pp_t,
            tc.tile_pool(name="pg", bufs=1, space=bass.MemorySpace.PSUM) as pp_g,
            tc.tile_pool(name="pa", bufs=4, space=bass.MemorySpace.PSUM) as pp_a,
            tc.tile_pool(name="pp", bufs=1, space=bass.MemorySpace.PSUM) as pp_p,
        ):
            # ------- static tiles
            ident = stat.tile([128, 128], F32, name="ident", tag="ident")
            make_identity(nc, ident[:])
            iota = stat.tile([128, 128], GDT, name="iota", tag="iota")
            nc.gpsimd.iota(iota[:], pattern=[[1, 128]], base=0,
                           channel_multiplier=0,
                           allow_small_or_imprecise_dtypes=True)
            w1_s = stat.tile([F, F], F32, name="w1s", tag="w1s")
            nc.sync.dma_start(w1_s[:], w1_d[:])
            w2_s = stat.tile([F, F], F32, name="w2s", tag="w2s")
            nc.sync.dma_start(w2_s[:], w2_d[:])
            wc_s = stat.tile([F, OD], F32, name="wcs", tag="wcs")
            nc.sync.dma_start(wc_s[:], wc_d[:])
            deg_s = stat.tile([128, NT], F32, name="degs", tag="degs")
            nc.sync.dma_start(deg_s[:], deg_d[:])
            bat_s = stat.tile([128, NT], F32, name="bats", tag="bats")
            nc.sync.dma_start(bat_s[:], bat_d[:])
            sqd = stat.tile([128, NT], F32, name="sqd", tag="sqd")
            nc.scalar.sqrt(sqd[:], deg_s[:])
            dinv = stat.tile([128, NT], F32, name="dinv", tag="dinv")
            nc.vector.reciprocal(dinv[:], sqd[:])
            ones = stat.tile([128, 1], F32, name="ones", tag="ones")
            nc.vector.memset(ones[:], 1.0)
            bias_s = []
            for use, bd, shape in ((use_b1, b1_d, [128, F]),
                                   (use_b2, b2_d, [128, F]),
                                   (use_bc, bc_d, [128, OD])):
                if use:
                    t = stat.tile(shape, F32, name=f"bs{len(bias_s)}",
                                  tag=f"bs{len(bias_s)}")
                    nc.sync.dma_start(t[:], bd[:])
                    bias_s.append(t)
                else:
                    bias_s.append(None)

            hA = [poolA.tile([128, F], F32, name=f"hA{j}", tag="hA")
                  for j in range(NT)]
            hB = [poolB.tile([128, F], F32, name=f"hB{j}", tag="hB")
                  for j in range(NT)]

            def layer(li, w_s, bias_t):
                # --- g = dinv * (h @ w); h source: x from DRAM (L1) or hB (L2)
                for j in range(NT):
                    if li == 0:
                        xj = poolM.tile([128, F], F32, name=f"x{li}_{j}",
                                        tag="xin")
                        nc.sync.dma_start(xj[:], x_d[j * 128:(j + 1) * 128, :])
                    else:
                        xj = hB[j]
                    ptr = pp_t.tile([128, 128], F32, name=f"tp{li}_{j}",
                                    tag="ptr")
                    nc.tensor.transpose(ptr[:], xj[:], ident[:])
                    xT = poolM.tile([128, F], F32, name=f"xT{li}_{j}", tag="xT")
                    nc.scalar.copy(xT[:], ptr[:])
                    pg = pp_g.tile([128, F], F32, name=f"pg{li}_{j}", tag="pg")
                    nc.tensor.matmul(pg[:], xT[:], w_s[:])
                    nc.scalar.activation(hA[j][:], pg[:],
                                         mybir.ActivationFunctionType.Copy,
                                         scale=dinv[:, j:j + 1])
                    if GDT is BF16:
                        src = poolM.tile([128, F], BF16, name=f"gbf{li}_{j}",
                                         tag="gbf")
                        nc.scalar.activation(src[:], pg[:],
                                             mybir.ActivationFunctionType.Copy,
                                             scale=dinv[:, j:j + 1])
                    else:
                        src = hA[j]
                    r_lo, r_hi = j * 128, j * 128 + 128
                    for q in range(r_lo // QSH, (r_hi - 1) // QSH + 1):
                        r0, r1 = max(r_lo, q * QSH), min(r_hi, (q + 1) * QSH)
                        nc.sync.dma_start(
                            gb_d[q][r0 - q * QSH:r1 - q * QSH, :],
                            src[r0 - r_lo:r1 - r_lo, :])
                # --- allgather the scaled shard into the 4 tables
                if not no_ag:
                    for q in range(Q):
                        nc.gpsimd.collective_compute(
                            "AllGather", mybir.AluOpType.bypass,
                            replica_groups=rg,
                            ins=[gb_d[q][:]],
                            outs=[tbl_d[q][:]],
                        )
                # --- gather + indicator-matmul aggregation, acc init = hA (self loop)
                for q in range(Q):
                    segs, by_tile = seg_info[q]
                    Lq = Lqs[q]
                    ntiles_q = Lq // 128
                    nchunks = (ntiles_q + CHT - 1) // CHT
                    chunk_cols = []
                    for ci in range(nchunks):
                        t0 = ci * CHT
                        nt_c = min(CHT, ntiles_q - t0)
                        cc = [c for k in range(t0, t0 + nt_c)
                              for (c, _D, _f, _l) in by_tile.get(k, [])]
                        chunk_cols.append((cc[0], cc[-1] + 1) if cc else (0, 0))
                    maxc = max(max((b - a for a, b in chunk_cols), default=1), 1)
                    cur_psum = {}
                    for ci in range(nchunks):
                        t0 = ci * CHT
                        ntile = min(CHT, ntiles_q - t0)
                        ntok = ntile * 128
                        idxt = poolL.tile([128, CH // 16], I16,
                                          name=f"ix{li}_{q}_{ci}", tag="idxt")
                        nc.sync.dma_start(
                            idxt[:, :ntok // 16],
                            idx_d[q][:, t0 * 8:t0 * 8 + ntok // 16])
                        c0, c1 = chunk_cols[ci]
                        relt = poolL.tile([128, maxc], F32,
                                          name=f"rl{li}_{q}_{ci}", tag="relt")
                        if c1 > c0:
                            nc.sync.dma_start(relt[:, :c1 - c0],
                                              rel_d[q][:, c0:c1])
                        msg = poolM.tile([128, CHT, F], GDT,
                                         name=f"mg{li}_{q}_{ci}", tag="msg")
                        if not no_gather:
                            nc.gpsimd.dma_gather(
                                msg[:, :ntile, :], tbl_d[q][:],
                                idxt[:, :ntok // 16], ntok, ntok, F,
                                queue_num=(ci % NQUEUES))
                        else:
                            # same-volume sequential DMA: ablates only the
                            # random-access/descriptor cost of the gather
                            nc.sync.dma_start(
                                msg[:, :ntile, :],
                                tbl_d[q][0:ntok, :].rearrange(
                                    "(c p) f -> p c f", p=128))
                        for k in range(t0, t0 + ntile):
                            for (col, D, first, last) in by_tile.get(k, []):
                                if no_ind and no_mm:
                                    continue
                                ind = poolI.tile([128, 128], GDT,
                                                 name=f"in{li}_{q}_{ci}_{col}",
                                                 tag="ind")
                                if not no_ind:
                                    nc.vector.tensor_scalar(
                                        ind[:], iota[:],
                                        relt[:, col - c0:col - c0 + 1],
                                        None, mybir.AluOpType.is_equal)
                                if no_mm:
                                    continue
                                if first:
                                    pa = pp_a.tile([128, F], F32,
                                                   name=f"pa{li}_{q}_{D}",
                                                   tag="pa")
                                    cur_psum[D] = pa
                                pa = cur_psum[D]
                                nc.tensor.matmul(pa[:], ind[:],
                                                 msg[:, k - t0, :],
                                                 start=first, stop=last)
                                if last:
                                    nc.vector.tensor_tensor(
                                        out=hA[D][:], in0=hA[D][:], in1=pa[:],
                                        op=mybir.AluOpType.add)
                                    del cur_psum[D]
                # --- finalize h = relu(dinv * acc (+ b))
                for j in range(NT):
                    if bias_t is None:
                        nc.scalar.activation(hB[j][:], hA[j][:],
                                             mybir.ActivationFunctionType.Relu,
                                             scale=dinv[:, j:j + 1])
                    else:
                        tmp = poolI.tile([128, F], F32, name=f"bt{li}_{j}",
                                         tag="ind")
                        nc.vector.tensor_scalar(tmp[:], hA[j][:],
                                                dinv[:, j:j + 1], None,
                                                mybir.AluOpType.mult)
                        nc.vector.tensor_tensor(out=tmp[:], in0=tmp[:],
                                                in1=bias_t[:],
                                                op=mybir.AluOpType.add)
                        nc.scalar.activation(hB[j][:], tmp[:],
                                             mybir.ActivationFunctionType.Relu)

            for _r in range(n_repeats):
                layer(0, w1_s, bias_s[0])
                layer(1, w2_s, bias_s[1])

            # ------- pooling: sums^T [feat, graph] and counts [graph, 1]
            ps = pp_p.tile([128, G], F32, name="psums", tag="psums")
            for j in range(NT):
                indg = poolI.tile([128, G], F32, name=f"ig{j}", tag="ind")
                nc.vector.tensor_scalar(indg[:], iota[:], bat_s[:, j:j + 1],
                                        None, mybir.AluOpType.is_equal)
                nc.tensor.matmul(ps[:], hB[j][:], indg[:],
                                 start=(j == 0), stop=(j == NT - 1))
            pn = pp_p.tile([128, 1], F32, name="pcnt", tag="pcnt")
            for j in range(NT):
                indg = poolI.tile([128, G], F32, name=f"ic{j}", tag="ind")
                nc.vector.tensor_scalar(indg[:], iota[:], bat_s[:, j:j + 1],
                                        None, mybir.AluOpType.is_equal)
                nc.tensor.matmul(pn[:], indg[:], ones[:],
                                 start=(j == 0), stop=(j == NT - 1))
            pack = stat.tile([128, F + 1], F32, name="pack", tag="pack")
            nc.scalar.copy(pack[:, 0:F], ps[:])
            nc.scalar.copy(pack[:, F:F + 1], pn[:])
            nc.sync.dma_start(ar_in_d[:], pack[:])
            nc.gpsimd.collective_compute(
                "AllReduce", mybir.AluOpType.add, replica_groups=rg,
                ins=[ar_in_d[:]], outs=[ar_out_d[:]])
            sums = stat.tile([128, F + 1], F32, name="sums", tag="sums")
            nc.sync.dma_start(sums[:], ar_out_d[:])
            cnt1 = stat.tile([128, 1], F32, name="cnt1", tag="cnt1")
            nc.vector.tensor_scalar_max(cnt1[:], sums[:, F:F + 1], 1.0)
            rcp = stat.tile([128, 1], F32, name="rcp", tag="rcp")
            nc.vector.reciprocal(rcp[:], cnt1[:])
            po = pp_g.tile([128, OD], F32, name="po", tag="pg")
            nc.tensor.matmul(po[:], sums[:, 0:F], wc_s[:])
            osb = stat.tile([128, OD], F32, name="osb", tag="osb")
            nc.scalar.activation(osb[:], po[:],
                                 mybir.ActivationFunctionType.Copy,
                                 scale=rcp[:])
            if bias_s[2] is not None:
                nc.vector.tensor_tensor(out=osb[:], in0=osb[:],
                                        in1=bias_s[2][:],
                                        op=mybir.AluOpType.add)
            nc.sync.dma_start(out_d[:], osb[:])

    nc.compile()
    return nc


# ------------------------------------------------------------------- driver

def run(inputs, cfg, trace=False):
    x = np.asarray(inputs["x"], dtype=np.float32)
    edge_index = np.asarray(inputs["edge_index"])
    batch = np.asarray(inputs["batch"])
    w1 = np.asarray(inputs["w1"], dtype=np.float32)
    b1 = np.asarray(inputs["b1"], dtype=np.float32)
    w2 = np.asarray(inputs["w2"], dtype=np.float32)
    b2 = np.asarray(inputs["b2"], dtype=np.float32)
    wc = np.asarray(inputs["wc"], dtype=np.float32)
    bc = np.asarray(inputs["bc"], dtype=np.float32)

    per_core, sched, Lqs = host_prep(x, edge_index, batch, cfg)
    nc = build_program(cfg, sched, Lqs, b1, b2, bc)

    in_maps = []
    for c in range(cfg["cores"]):
        pc = per_core[c]
        m = dict(x=pc["x"], deg=pc["deg"], bat=pc["bat"],
                 w1=w1, w2=w2, wc=wc)
        for q in range(cfg["q"]):
            m[f"idx{q}"] = pc["idx"][q]
            m[f"rel{q}"] = pc["rel"][q]
        if np.any(b1):
            m["b1b"] = np.tile(b1[None, :], (128, 1)).astype(np.float32)
        if np.any(b2):
            m["b2b"] = np.tile(b2[None, :], (128, 1)).astype(np.float32)
        if np.any(bc):
            m["bcb"] = np.tile(bc[None, :], (128, 1)).astype(np.float32)
        in_maps.append(m)

    res = run_bass_kernel_spmd(nc, in_maps, list(range(cfg["cores"])),
                               trace=trace)
    out = np.asarray(res.results[0]["out"], dtype=np.float32)
    return out, res


def kernel(**inputs) -> np.ndarray:
    out, _ = run(inputs, default_cfg(), trace=False)
    return out

